# revision 31
# baseline (speedup 1.0000x reference)
"""Trainium2 Bass kernel for BitNet-style cross-attention (8 NeuronCores).

Data-parallel token sharding: b=2, n=2048 -> 4096 query-token rows; each of
the 8 cores owns 512 (cores 0-3 batch 0, 4-7 batch 1) and computes its output
slice independently (k/v recomputed per core).

Key ideas vs the naive formulation:
 - Per-token activation-quant scales factor out of every GEMM, so activations
   are matmul'd as raw int8-valued bf16 and all dequant scales are folded
   into PSUM evictions / the softmax exp (per-partition Act-engine scale
   operands; per-ctx-token scales obtained via tiny PE transposes).
 - round() via the fp32 magic-constant trick ((v+1.5*2^23)-1.5*2^23, exact
   round-half-even) in fused DVE tensor_scalar ops -- no int8 round-trips,
   no GpSimd casts.
 - The attention is STREAMED over the two ctx halves: scores/exp/attn@v for
   half 0 run while half 1 is still being DMA'd/quantized/projected, hiding
   most of the Act-engine exp cost (the single largest fixed cost) under the
   PE-heavy projection phase.  Attention output accumulates unnormalized in
   SBUF; the softmax denominator (also accumulated) is applied once at the
   end, where it cancels inside the output act-quant.
"""

import numpy as np

import concourse.bass as bass
import concourse.mybir as mybir
import concourse.tile as tile
from concourse import bacc, bass_isa
from concourse.bass_utils import run_bass_kernel_spmd

F32 = mybir.dt.float32
BF16 = mybir.dt.bfloat16
AX = mybir.AxisListType
OP = mybir.AluOpType
AF = mybir.ActivationFunctionType

P = 128
MAGIC = 12582912.0  # 1.5 * 2**23: fp32 add/sub rounds to nearest int (ties even)

CFG_FULL = dict(DIM=1024, INNER=1024, H=16, D=64, NTOK=512, MCTX=2048)
N_CORES = 8
EPS = 1e-5


def build(cfg):
    DIM, INNER, H, D = cfg["DIM"], cfg["INNER"], cfg["H"], cfg["D"]
    NTOK, MCTX = cfg["NTOK"], cfg["MCTX"]
    KC = DIM // P            # input-dim 128-chunks (8)
    IC = INNER // P          # inner-dim 128-chunks (8)
    NKB = MCTX // P          # ctx 128-blocks (16)
    NTB = NTOK // P          # query-token 128-blocks (4)
    QTOK = 512               # K-proj moving width
    ETOK = 256               # ctx staging eighth size
    NE = MCTX // ETOK        # 8 eighths
    EKB = ETOK // P          # ctx 128-blocks per eighth (2)
    HKB = NKB // 2           # ctx 128-blocks per half (8)
    VW = D + 1               # v columns per head incl ones
    HPH = (INNER // 2) // D  # heads per inner half (8)

    nc = bacc.Bacc("TRN2", target_bir_lowering=False, debug=False,
                   num_devices=N_CORES)

    xT = nc.dram_tensor("xT", [DIM, NTOK], F32, kind="ExternalInput")
    cT = nc.dram_tensor("cT", [DIM, MCTX], F32, kind="ExternalInput")
    wT = {}
    for w in ("wq", "wk", "wv", "wo"):
        wT[w] = nc.dram_tensor(w + "T", [DIM, INNER], F32, kind="ExternalInput")
    iden = nc.dram_tensor("iden", [P, P], F32, kind="ExternalInput")
    y_out = nc.dram_tensor("y", [NTOK, DIM], F32, kind="ExternalOutput")

    from contextlib import ExitStack
    with tile.TileContext(nc) as tc, ExitStack() as ctx:
        # ---- pools ------------------------------------------------------
        pp = ctx.enter_context(tc.tile_pool(name="persist", bufs=1))
        smp = ctx.enter_context(tc.tile_pool(name="small", bufs=1))
        asp = ctx.enter_context(tc.tile_pool(name="astage", bufs=1))

        # shared PSUM pools (8 banks total across ALL phases):
        #   ps_sv: [128,2,512]x2 = 4 banks (attention ss + V-proj pv)
        #   ps_ka: [128,512]x2   = 2 banks (pq/pk/py/pt)
        #   ps_po: [65,512]x2    = 2 banks (attention po accumulators)
        ps_sv = ctx.enter_context(tc.tile_pool(name="ps_sv", bufs=2,
                                               space="PSUM"))
        ps_ka = ctx.enter_context(tc.tile_pool(name="ps_ka", bufs=2,
                                               space="PSUM"))
        ps_po = ctx.enter_context(tc.tile_pool(name="ps_po", bufs=2,
                                               space="PSUM"))

        qb = pp.tile([P, IC, NTOK], BF16, tag="qb")     # q*inv_x, feat-major
        kb = pp.tile([P, IC, MCTX], BF16, tag="kb")     # k raw ints, feat-major
        vb = pp.tile([P, NKB * H * VW], BF16, tag="vb")  # v natural + ones col
        vb3 = vb[:].rearrange("p (k h w) -> p k h w", h=H, w=VW)
        idt = pp.tile([P, P], F32, tag="idt")           # identity for PE transp
        nc.sync.dma_start(out=idt[:], in_=iden.ap()[:, :])
        icT = pp.tile([P, NKB], F32, tag="icT")         # inv_c, ctx-token-major
        vsc = pp.tile([P, NKB], F32, tag="vsc")         # icT * mean|wv|
        esc = pp.tile([P, NKB], F32, tag="esc")         # icT * mq*mk/sqrt(D)

        wmean = {}

        # ---- weight quantization ----------------------------------------
        def quant_weight(w, wsp, dst_pool, tern_eng="act"):
            ws = wsp.tile([P, KC * INNER], F32, tag="wst")
            for c in range(KC):
                nc.sync.dma_start(out=ws[:, c * INNER:(c + 1) * INNER],
                                  in_=wT[w].ap()[c * P:(c + 1) * P, :])
            wbt = dst_pool.tile([P, KC * INNER], BF16, tag="wb_" + w,
                                name="wb_" + w)
            wsum = smp.tile([P, 1], F32, tag="wsum")
            nc.scalar.activation(wbt[:], ws[:], AF.Abs, accum_out=wsum[:])
            wrep = smp.tile([P, 1], F32, tag="wrep")
            nc.gpsimd.partition_all_reduce(wrep[:], wsum[:], channels=P,
                                           reduce_op=bass_isa.ReduceOp.add)
            mean = smp.tile([P, 1], F32, tag="wmean_" + w, name="mean_" + w)
            nc.vector.tensor_scalar(mean[:], wrep[:], 1.0 / (DIM * INNER),
                                    EPS, OP.mult, OP.max)
            qs = smp.tile([P, 1], F32, tag="wqs_" + w, name="qs_" + w)
            nc.vector.reciprocal(qs[:], mean[:])
            wmean[w] = mean
            nc.vector.tensor_scalar(ws[:], ws[:], qs[:], 1.49, OP.mult, OP.min)
            nc.vector.tensor_scalar(ws[:], ws[:], -1.49, MAGIC, OP.max, OP.add)
            if tern_eng == "act":
                nc.scalar.activation(wbt[:], ws[:], AF.Copy, bias=-MAGIC)
            else:
                nc.vector.tensor_scalar(wbt[:], ws[:], -MAGIC, None, OP.add)
            return wbt

        # wo variant: quarter-staged with re-DMA (small SBUF footprint so it
        # can run during the attention phase)
        def quant_weight_wo(wsp, dst_pool):
            w = "wo"
            NWQ = 4
            CPQ = KC // NWQ  # chunks per quarter (2)
            WQW = CPQ * INNER
            wbt = dst_pool.tile([P, KC * INNER], BF16, tag="wb_wo",
                                name="wb_wo")
            wsums = smp.tile([P, NWQ], F32, tag="wsums")
            for i in range(NWQ):
                ws = wsp.tile([P, WQW], F32, tag="wstq", name=f"woA{i}")
                for c in range(CPQ):
                    cc = i * CPQ + c
                    nc.sync.dma_start(out=ws[:, c * INNER:(c + 1) * INNER],
                                      in_=wT[w].ap()[cc * P:(cc + 1) * P, :])
                nc.scalar.activation(wbt[:, i * WQW:(i + 1) * WQW], ws[:],
                                     AF.Abs, accum_out=wsums[:, i:i + 1])
            wsum = smp.tile([P, 1], F32, tag="wsum")
            nc.vector.tensor_reduce(wsum[:], wsums[:], axis=AX.X, op=OP.add)
            wrep = smp.tile([P, 1], F32, tag="wrep")
            nc.gpsimd.partition_all_reduce(wrep[:], wsum[:], channels=P,
                                           reduce_op=bass_isa.ReduceOp.add)
            mean = smp.tile([P, 1], F32, tag="wmean_wo", name="mean_wo")
            nc.vector.tensor_scalar(mean[:], wrep[:], 1.0 / (DIM * INNER),
                                    EPS, OP.mult, OP.max)
            qs = smp.tile([P, 1], F32, tag="wqs_wo", name="qs_wo")
            nc.vector.reciprocal(qs[:], mean[:])
            wmean[w] = mean
            for i in range(NWQ):
                ws = wsp.tile([P, WQW], F32, tag="wstq", name=f"woB{i}")
                for c in range(CPQ):
                    cc = i * CPQ + c
                    nc.sync.dma_start(out=ws[:, c * INNER:(c + 1) * INNER],
                                      in_=wT[w].ap()[cc * P:(cc + 1) * P, :])
                nc.vector.tensor_scalar(ws[:], ws[:], qs[:], 1.49,
                                        OP.mult, OP.min)
                nc.vector.tensor_scalar(ws[:], ws[:], -1.49, MAGIC,
                                        OP.max, OP.add)
                nc.vector.tensor_scalar(wbt[:, i * WQW:(i + 1) * WQW], ws[:],
                                        -MAGIC, None, OP.add)
            return wbt

        # ---- activation quantization (feature-major, no dequant) --------
        def act_quant(src, dst, dcol0, inv_rep, ncol):
            amax = asp.tile([P, ncol], F32, tag="amax")
            nc.vector.tensor_reduce(
                amax[:], src.rearrange("p c t -> p t c"),
                axis=AX.X, op=OP.max, apply_absolute_value=True)
            rep = asp.tile([P, ncol], F32, tag="arep")
            nc.gpsimd.partition_all_reduce(rep[:], amax[:], channels=P,
                                           reduce_op=bass_isa.ReduceOp.max)
            nc.vector.tensor_scalar(inv_rep, rep[:], EPS, 1.0 / 127.0,
                                    OP.max, OP.mult)
            rq = asp.tile([P, ncol], F32, tag="arq")
            nc.vector.reciprocal(rq[:], inv_rep)
            for c in range(KC):
                tmp = asp.tile([P, ncol], F32, tag="atmp")
                nc.vector.tensor_tensor(tmp[:], src[:, c, :], rq[:],
                                        op=OP.mult)
                nc.vector.tensor_scalar(
                    dst[:, c, dcol0:dcol0 + ncol], tmp[:], MAGIC, -MAGIC,
                    OP.add, OP.add)

        def ctx_dma(e):
            cs = csp.tile([P, KC, ETOK], F32, tag="cs", name=f"cs{e}")
            col0 = e * ETOK
            for c in range(KC):
                nc.sync.dma_start(
                    out=cs[:, c, :],
                    in_=cT.ap()[c * P:(c + 1) * P, col0:col0 + ETOK])
            return cs

        def ctx_quant(e, cs, cdq):
            inv_c = csp.tile([P, ETOK], F32, tag="invc", name=f"invc{e}")
            lcol = (e % (NE // 2)) * ETOK
            act_quant(cs[:], cdq[:], lcol, inv_c[:], ETOK)
            for kk in range(EKB):
                kbk = e * EKB + kk
                pt = ps_ka.tile([P, P], F32, tag="pka", name=f"pt{kbk}")
                nc.tensor.transpose(pt[:], inv_c[:, kk * P:(kk + 1) * P],
                                    idt[:])
                nc.scalar.copy(icT[:, kbk:kbk + 1], pt[:, 0:1])

        def k_proj(q, wkb3, cdq):
            lcol = (q % 2) * QTOK
            for ic in range(IC):
                pk = ps_ka.tile([P, QTOK], F32, tag="pka", name=f"pk{q}_{ic}")
                for c in range(KC):
                    nc.tensor.matmul(
                        pk[:], wkb3[:, c, ic * P:(ic + 1) * P],
                        cdq[:, c, lcol:lcol + QTOK],
                        start=(c == 0), stop=(c == KC - 1))
                nc.scalar.copy(kb[:, ic, q * QTOK:(q + 1) * QTOK], pk[:])

        def v_proj(half, wvb3, cdq):
            for kk in range(HKB):
                kbk = half * HKB + kk
                pv = ps_sv.tile([P, 2, INNER // 2], F32, tag="psv",
                                name=f"pv{kbk}")
                for c in range(KC):
                    for ih in range(2):
                        nc.tensor.matmul(
                            pv[:, ih, :],
                            cdq[:, c, kk * P:(kk + 1) * P],
                            wvb3[:, c, ih * (INNER // 2):
                                 (ih + 1) * (INNER // 2)],
                            start=(c == 0), stop=(c == KC - 1))
                for ih in range(2):
                    nc.scalar.mul(
                        vb3[:, kbk, ih * HPH:(ih + 1) * HPH, 0:D],
                        pv[:, ih, :].rearrange("p (h d) -> p h d", d=D),
                        vsc[:, kbk:kbk + 1])

        def scales_for_half(half, qkm):
            sl = slice(half * HKB, (half + 1) * HKB)
            nc.vector.tensor_scalar(vsc[:, sl], icT[:, sl],
                                    wmean["wv"][:], None, OP.mult)
            nc.vector.tensor_scalar(esc[:, sl], icT[:, sl], qkm[:], None,
                                    OP.mult)

        def attn_half(half, after_hp=None):
            # streamed attention over one ctx half; accumulates unnormalized
            # numerators into otT and denominators into dnacc.  after_hp maps
            # hp -> callable emitting extra PE work interleaved into the
            # (Act-paced) attention stream.
            for hp in range(H // 2):
                hA, hB = 2 * hp, 2 * hp + 1
                pA, pB = (hA * D) % P, (hB * D) % P
                po = [ps_po.tile([VW, NTOK], F32, tag="po",
                                 name=f"po{half}_{hp}_{j}") for j in range(2)]
                for kk in range(HKB):
                    kbk = half * HKB + kk
                    ss = ps_sv.tile([P, 2, NTOK], F32, tag="psv", name="ss")
                    for j, (h, ph) in enumerate([(hA, pA), (hB, pB)]):
                        nc.tensor.matmul(
                            ss[:, j, :],
                            kb[ph:ph + D, hp, kbk * P:(kbk + 1) * P],
                            qb[ph:ph + D, hp, :],
                            start=True, stop=True)
                    et = ep.tile([P, 2, NTOK], BF16, tag="et")
                    nc.scalar.activation(et[:], ss[:], AF.Exp,
                                         scale=esc[:, kbk:kbk + 1])
                    for j, h in enumerate((hA, hB)):
                        nc.tensor.matmul(
                            po[j][0:VW, :],
                            vb3[:, kbk, h, :],
                            et[:, j, :],
                            start=(kk == 0), stop=(kk == HKB - 1))
                if half == 0:
                    for j, (h, ph) in enumerate([(hA, pA), (hB, pB)]):
                        k, r = h // 4, 32 * (h % 4)
                        nc.vector.tensor_copy(otT[ph:ph + D, hp, :],
                                              po[j][0:D, :])
                        nc.vector.tensor_copy(dnacc4[k][r:r + 1, :],
                                              po[j][D:D + 1, :])
                else:
                    # accumulate numerators (aligned via unary-copy staging;
                    # tensor_tensor inputs must share a start partition) and
                    # finish the denominator sums; normalization happens in
                    # a batched pass after the loop.
                    for j, (h, ph) in enumerate([(hA, pA), (hB, pB)]):
                        k, r = h // 4, 32 * (h % 4)
                        dts = rbp.tile([P, NTOK], F32, tag="dts")
                        nc.vector.tensor_copy(dts[r:r + 1, :],
                                              po[j][D:D + 1, :])
                        nc.vector.tensor_tensor(dnacc4[k][r:r + 1, :],
                                                dnacc4[k][r:r + 1, :],
                                                dts[r:r + 1, :], op=OP.add)
                        pstg = rbp.tile([P, NTOK], F32, tag="scr",
                                        name=f"pstg{hp}_{j}")
                        nc.vector.tensor_copy(pstg[ph:ph + D, :],
                                              po[j][0:D, :])
                        nc.vector.tensor_tensor(otT[ph:ph + D, hp, :],
                                                otT[ph:ph + D, hp, :],
                                                pstg[ph:ph + D, :],
                                                op=OP.add)
                if after_hp and hp in after_hp:
                    after_hp[hp]()
            if half == 1:
                # batched normalize: 4 reciprocals cover all 16 denominators
                for k in range(4):
                    nc.vector.reciprocal(dnacc4[k][:], dnacc4[k][:])
                for hp in range(H // 2):
                    for j in range(2):
                        h = 2 * hp + j
                        ph = (h * D) % P
                        k, r = h // 4, 32 * (h % 4)
                        rd1 = rbp.tile([1, NTOK], F32, tag="rd1")
                        nc.vector.tensor_copy(rd1[:], dnacc4[k][r:r + 1, :])
                        rbal = rbp.tile([P, NTOK], F32, tag="scr",
                                        name=f"rbal{hp}_{j}")
                        nc.gpsimd.partition_broadcast(rbal[0:D, :], rd1[:])
                        if ph != 0:
                            nc.vector.tensor_copy(rbal[ph:ph + D, :],
                                                  rbal[0:D, :])
                        nc.vector.tensor_tensor(otT[ph:ph + D, hp, :],
                                                otT[ph:ph + D, hp, :],
                                                rbal[ph:ph + D, :],
                                                op=OP.mult)

        # ================= emission =====================================
        csp_cm = tc.tile_pool(name="cstage", bufs=2)
        csp = csp_cm.__enter__()
        with tc.tile_pool(name="wstage", bufs=1) as wsp:
            # front-load x + wq DMA, then first ctx eighths
            with tc.tile_pool(name="xq", bufs=1) as xqp, \
                    tc.tile_pool(name="xstage", bufs=1) as xsp, \
                    tc.tile_pool(name="wbqq", bufs=1) as wbpq:
                xs = xsp.tile([P, KC, NTOK], F32, tag="xs")
                for c in range(KC):
                    nc.sync.dma_start(out=xs[:, c, :],
                                      in_=xT.ap()[c * P:(c + 1) * P, :])
                wqb = quant_weight("wq", wsp, wbpq, tern_eng="act")
                cs0 = ctx_dma(0)
                cs1 = ctx_dma(1)
                xdq = xqp.tile([P, KC, NTOK], BF16, tag="xdq")
                inv_x = xqp.tile([P, NTOK], F32, tag="invx")
                act_quant(xs[:], xdq[:], 0, inv_x[:], NTOK)

                # Q projection
                wqb3 = wqb[:].rearrange("p (c i) -> p c i", c=KC)
                for ic in range(IC):
                    pq = ps_ka.tile([P, NTOK], F32, tag="pka", name=f"pq{ic}")
                    for c in range(KC):
                        nc.tensor.matmul(
                            pq[:], wqb3[:, c, ic * P:(ic + 1) * P],
                            xdq[:, c, :],
                            start=(c == 0), stop=(c == KC - 1))
                    nc.vector.tensor_tensor(qb[:, ic, :], pq[:], inv_x[:],
                                            op=OP.mult)

            with tc.tile_pool(name="cq0", bufs=1) as cq0p:
                cdq0 = cq0p.tile([P, KC, MCTX // 2], BF16, tag="cdq0")
                ctx_quant(0, cs0, cdq0)
                ctx_quant(1, cs1, cdq0)
                for e in (2, 3):
                    cs = ctx_dma(e)
                    ctx_quant(e, cs, cdq0)

                wbpk = ctx.enter_context(tc.tile_pool(name="wbqk", bufs=1,
                                                      side="right"))
                wkb = quant_weight("wk", wsp, wbpk, tern_eng="dve")
                wkb3 = wkb[:].rearrange("p (c i) -> p c i", c=KC)
                wbpv = ctx.enter_context(tc.tile_pool(name="wbqv", bufs=1,
                                                      side="right"))
                wvb = quant_weight("wv", wsp, wbpv, tern_eng="dve")
                wvb3 = wvb[:].rearrange("p (c i) -> p c i", c=KC)

                qkm = smp.tile([P, 1], F32, tag="qkm")
                nc.vector.tensor_tensor(qkm[:], wmean["wq"][:],
                                        wmean["wk"][:], op=OP.mult)
                nc.vector.tensor_scalar(qkm[:], qkm[:],
                                        1.0 / float(np.sqrt(D)), None,
                                        OP.mult)
                nc.vector.memset(vb3[:, :, :, D], 1.0)
                scales_for_half(0, qkm)

                k_proj(0, wkb3, cdq0)
                k_proj(1, wkb3, cdq0)
                v_proj(0, wvb3, cdq0)

        # half-1 staging DMA+quant overlaps half-0 attention; K23/V1 PE
        # work is interleaved into the (Act-paced) half-0 attention stream
        op_pool = ctx.enter_context(tc.tile_pool(name="opool", bufs=1,
                                                  side="right"))
        otT = op_pool.tile([P, IC, NTOK], F32, tag="otT")  # attn num accum
        # den accumulators packed at partition offsets {0,32,64,96} (engine
        # APs may only start at multiples of 32); den d=2hp+j -> tile d//4,
        # row 32*(d%4)
        dnacc4 = [op_pool.tile([P, NTOK], F32, tag=f"dnacc{k}",
                                name=f"dnacc{k}") for k in range(4)]
        for k in range(4):
            nc.vector.memset(dnacc4[k][:], 1.0)
        ep = ctx.enter_context(tc.tile_pool(name="etile", bufs=3,
                                             side="right"))
        rbp = ctx.enter_context(tc.tile_pool(name="rbpool", bufs=1,
                                             side="right"))
        with tc.tile_pool(name="cq1", bufs=1) as cq1p:
            cdq1 = cq1p.tile([P, KC, MCTX // 2], BF16, tag="cdq1")
            for e in (4, 5, 6, 7):
                cs = ctx_dma(e)
                ctx_quant(e, cs, cdq1)
            scales_for_half(1, qkm)

            attn_half(0, after_hp={
                1: lambda: k_proj(2, wkb3, cdq1),
                3: lambda: k_proj(3, wkb3, cdq1),
                5: lambda: v_proj(1, wvb3, cdq1),
            })

        csp_cm.__exit__(None, None, None)

        # wo quant (small-footprint, overlaps attention tail)
        wop = ctx.enter_context(tc.tile_pool(name="wopool", bufs=1))
        with tc.tile_pool(name="wstage2", bufs=2) as wsp2:
            wob = quant_weight_wo(wsp2, wop)

        attn_half(1)

        # ---- attn-out quantization + output projection ------------------
        with tc.tile_pool(name="oq", bufs=2) as oqp, \
                tc.tile_pool(name="ysb", bufs=2) as yp:
            odq = op_pool.tile([P, IC, NTOK], BF16, tag="odq")
            inv_o = op_pool.tile([P, NTOK], F32, tag="invo")
            act_quant(otT[:], odq[:], 0, inv_o[:], NTOK)

            # y-eviction scale, token-major: syT = (inv_o).T * mean|wo|
            syT = smp.tile([P, NTB], F32, tag="syT")
            for tb in range(NTB):
                pt = ps_ka.tile([P, P], F32, tag="pka", name=f"pt2{tb}")
                nc.tensor.transpose(pt[:], inv_o[:, tb * P:(tb + 1) * P],
                                    idt[:])
                nc.scalar.copy(syT[:, tb:tb + 1], pt[:, 0:1])
            nc.vector.tensor_scalar(syT[:], syT[:], wmean["wo"][:], None,
                                    OP.mult)

            wob3 = wob[:].rearrange("p (c i) -> p c i", c=IC)
            for tb in range(NTB):
                for oh in range(2):
                    py = ps_ka.tile([P, DIM // 2], F32, tag="pka",
                                    name=f"py{tb}_{oh}")
                    for c in range(IC):
                        nc.tensor.matmul(
                            py[:],
                            odq[:, c, tb * P:(tb + 1) * P],
                            wob3[:, c, oh * (DIM // 2):(oh + 1) * (DIM // 2)],
                            start=(c == 0), stop=(c == IC - 1))
                    ysb = yp.tile([P, DIM // 2], F32, tag="ysb")
                    nc.scalar.mul(ysb[:], py[:], syT[:, tb:tb + 1])
                    nc.sync.dma_start(
                        out=y_out.ap()[tb * P:(tb + 1) * P,
                                       oh * (DIM // 2):(oh + 1) * (DIM // 2)],
                        in_=ysb[:])
    nc.compile()
    return nc


_CACHE = {}


def _get_nc(key, cfg):
    if key not in _CACHE:
        _CACHE[key] = build(cfg)
    return _CACHE[key]


def _shard(x, context, wq, wk, wv, wo, NTOK):
    b = x.shape[0]
    wmaps = {w + "T": np.ascontiguousarray(a.T)
             for w, a in (("wq", wq), ("wk", wk), ("wv", wv), ("wo", wo))}
    wmaps["iden"] = np.eye(128, dtype=np.float32)
    cores_per_b = N_CORES // b
    in_maps = []
    for core in range(N_CORES):
        bi = core // cores_per_b
        t0 = (core % cores_per_b) * NTOK
        in_maps.append(dict(
            xT=np.ascontiguousarray(x[bi, t0:t0 + NTOK, :].T),
            cT=np.ascontiguousarray(context[bi].T),
            **wmaps))
    return in_maps


def _assemble(results, b, n, dim, NTOK):
    out = np.empty((b, n, dim), dtype=np.float32)
    cores_per_b = N_CORES // b
    for core in range(N_CORES):
        bi = core // cores_per_b
        t0 = (core % cores_per_b) * NTOK
        out[bi, t0:t0 + NTOK, :] = results[core]["y"]
    return out


def run(x, context, wq, wk, wv, wo, trace=False):
    cfg = CFG_FULL
    b, n, dim = x.shape
    NTOK = cfg["NTOK"]
    nc = _get_nc("full", cfg)
    in_maps = _shard(x, context, wq, wk, wv, wo, NTOK)
    res = run_bass_kernel_spmd(nc, in_maps, list(range(N_CORES)), trace=trace)
    return _assemble(res.results, b, n, dim, NTOK), res


def kernel(x, context, wq, wk, wv, wo):
    return run(x, context, wq, wk, wv, wo, trace=False)[0]


if __name__ == "__main__":
    ins = {k: np.random.randn(*s).astype(np.float32) * (0.02 if k[0] == 'w' else 1.0)
           for k, s in [("x", (2, 2048, 1024)), ("context", (2, 2048, 1024)),
                        ("wq", (1024, 1024)), ("wk", (1024, 1024)),
                        ("wv", (1024, 1024)), ("wo", (1024, 1024))]}
    y = kernel(**ins)
    print("kernel output", y.shape, y.dtype, np.abs(y).max())
